# revision 32
# baseline (speedup 1.0000x reference)
"""Trainium2 Bass kernel: aspect-level sentiment classification head.

  aspect[b] = mean(last_hidden_state[b, start_b:end_b, :])   (ragged spans)
  out = concat([pooled, aspect], -1) @ W.T + b

Strategy: data-parallel over batch with host-side load balancing.  Samples
are assigned to cores (8 per core) so the per-core total span length is
minimized; each core gathers exactly its spans' rows (tightly packed, no
per-sample padding) from DRAM with G indirect DMAs of 128 rows each.  All
index/mask arithmetic happens on the host: the kernel receives ready-made
gather indices plus a bf16 "weight mask" whose entries are 1/len placed at
(row, sample) positions, so a single bf16 matmul per 128-column hidden chunk
produces the *transposed* aspect features directly.  Pooled features and W
arrive host-pre-transposed in one bf16 blob; the pooled half of the final
GEMM is issued before the gather completes so it overlaps the DMA.

HW-validated details (probed on trn2; CoreSim is more permissive):
  - indirect-DMA offset APs must be partition-major single columns
    ([128, 1] works, [1, 128] and [16, 8] crash the exec unit);
  - the SWDGE gather may cast f32 -> bf16 in flight;
  - single_packet=True on the [128, G] idx load cuts its HWDGE
    descriptor generation from 128 descriptors to one packet;
  - splitting the gather loses: SWDGE generation is ~1us fixed per
    instruction, dwarfing the transfer-overlap it buys.
"""

import sys

if "/opt/trn_rl_repo" not in sys.path:
    sys.path.insert(0, "/opt/trn_rl_repo")

import numpy as np
import ml_dtypes

import concourse.tile as tile
from concourse import bacc, mybir
from concourse.bass import IndirectOffsetOnAxis
from concourse.bass_utils import run_bass_kernel_spmd

F32 = mybir.dt.float32
BF16 = mybir.dt.bfloat16
I32 = mybir.dt.int32

B, S, H, C = 64, 4096, 768, 3
NCORES = 8
BL = B // NCORES          # samples per core
P = 128
HC = H // P               # 6 hidden chunks of 128
KC = 2 * H // P           # 12 contraction chunks in the final GEMM

# blob column layout (all bf16): pT | wT | wmask | ones8 | bias
PT0, WT0 = 0, HC * BL                        # 0, 48
def _cols(G):
    wm0 = WT0 + KC * C                       # 84
    on0 = wm0 + BL * G
    b0 = on0 + BL
    return wm0, on0, b0, b0 + C


# partition-id tensor off for HW (saves a ~1.3us register load in the
# preamble); the CPU-sim path needs it on, so sim harnesses set PID = True
PID = False
# number of dummy PE matmuls issued while the gather is in flight (HAM
# warmup).  Measured ineffective at small counts (they pipeline at ~25ns
# each, far below the ~3.4us of activity the HAM window needs), so off.
WARMUP = 0


def build(G: int, with_bias: bool = True):
    """Per-core SPMD program gathering G*128 tightly packed span rows."""
    WM0, ON0, B0, BW = _cols(G)
    nc = bacc.Bacc("TRN2", target_bir_lowering=False, debug=False,
                   num_devices=NCORES, enable_partition_id=PID)
    lhs = nc.dram_tensor("lhs", [BL * S, H], F32, kind="ExternalInput").ap()
    idx = nc.dram_tensor("idx", [P, G], I32, kind="ExternalInput").ap()
    blob = nc.dram_tensor("blob", [P, BW], BF16, kind="ExternalInput").ap()
    out = nc.dram_tensor("out", [BL, C], F32, kind="ExternalOutput").ap()

    with tile.TileContext(nc) as tc:
        with (
            tc.tile_pool(name="const", bufs=1) as cp,
            tc.tile_pool(name="rows", bufs=min(2, G)) as rp,
            tc.tile_pool(name="pout", bufs=1, space="PSUM") as po,
            tc.tile_pool(name="pasp", bufs=1, space="PSUM") as pa,
        ):
            # idx on Sync HWDGE: measured faster than SWDGE-on-Pool (which
            # serializes idx gen + gather gen on the same Q7)
            idx_sb = cp.tile([P, G], I32, tag="idx_sb")
            nc.sync.dma_start(idx_sb[:], idx[:, :], single_packet=True)
            blob_sb = cp.tile([P, BW], BF16, tag="blob_sb")
            nc.scalar.dma_start(blob_sb[:], blob[:, :])
            pT = blob_sb[:, PT0:PT0 + HC * BL]
            wT = blob_sb[:, WT0:WT0 + KC * C]

            # pooled half of the final GEMM runs while the gather is in
            # flight; the accumulation group stays open until the bias term
            out_ps = po.tile([BL, C], F32, tag="out_ps")
            for c in range(HC):
                nc.tensor.matmul(out=out_ps[:], lhsT=pT[:, c * BL:(c + 1) * BL],
                                 rhs=wT[:, c * C:(c + 1) * C],
                                 start=(c == 0), stop=False)

            # keep PE busy through the gather wait so the HAM window ramps
            # the clock to 2.4GHz before the real matmuls arrive
            if WARMUP:
                warm_ps = po.tile([BL, C], F32, tag="warm_ps",
                                  name="warm_ps")
                for _ in range(WARMUP):
                    nc.tensor.matmul(out=warm_ps[:],
                                     lhsT=pT[:, 0:BL], rhs=wT[:, 0:C],
                                     start=True, stop=True)

            # aspT[h, s] = sum_r rows[r, h] * wmask[r, s].  For G == 1 all
            # chunks share one PSUM bank and one PSUM->SBUF cast (DVE
            # per-instruction overhead beats per-chunk pipelining); for
            # G > 1 the accumulation groups stay open across gathers, so
            # each chunk gets its own bank.
            if G == 1:
                aspT_all = pa.tile([P, HC * BL], F32, tag="aspT")
                aspT_ps = [aspT_all[:, c * BL:(c + 1) * BL]
                           for c in range(HC)]
            else:
                aspT_ps = [pa.tile([P, BL], F32, tag=f"aspT{c}",
                                   name=f"aspT{c}")[:] for c in range(HC)]
            for g in range(G):
                # SWDGE casts f32 -> bf16 during the gather (probed on HW)
                rows_b = rp.tile([P, H], BF16, tag="rows_b")
                nc.gpsimd.indirect_dma_start(
                    out=rows_b[:], out_offset=None, in_=lhs[:, :],
                    in_offset=IndirectOffsetOnAxis(
                        ap=idx_sb[:, g:g + 1], axis=0))
                wm_g = blob_sb[:, WM0 + g * BL:WM0 + (g + 1) * BL]
                for c in range(HC):
                    nc.tensor.matmul(out=aspT_ps[c][:, :],
                                     lhsT=rows_b[:, c * P:(c + 1) * P],
                                     rhs=wm_g, start=(g == 0), stop=(g == G - 1))

            aspT_sb = cp.tile([P, HC * BL], BF16, tag="aspT_sb")
            if G == 1:
                nc.vector.tensor_copy(aspT_sb[:], aspT_all[:])
            else:
                for c in range(HC):
                    nc.vector.tensor_copy(aspT_sb[:, c * BL:(c + 1) * BL],
                                          aspT_ps[c])
            for c in range(HC):
                nc.tensor.matmul(out=out_ps[:],
                                 lhsT=aspT_sb[:, c * BL:(c + 1) * BL],
                                 rhs=wT[:, (HC + c) * C:(HC + c + 1) * C],
                                 start=False,
                                 stop=(not with_bias and c == HC - 1))
            if with_bias:
                # bias as a rank-1 accumulation: ones[1,8].T @ bias_row[1,3]
                nc.tensor.matmul(out=out_ps[:],
                                 lhsT=blob_sb[0:1, ON0:ON0 + BL],
                                 rhs=blob_sb[0:1, B0:B0 + C], start=False,
                                 stop=True)

            out_sb = cp.tile([BL, C], F32, tag="out_sb")
            nc.vector.tensor_copy(out_sb[:], out_ps[:])
            nc.sync.dma_start(out[:, :], out_sb[:], single_packet=True)

    nc.compile()
    return nc


_CACHE: dict = {}


def _get(G: int, with_bias: bool = True):
    key = (G, PID, with_bias, WARMUP)
    if key not in _CACHE:
        _CACHE[key] = build(G, with_bias)
    return _CACHE[key]


def _assign(lens: np.ndarray) -> list[list[int]]:
    """Assign samples to cores, BL per core, minimizing max sum(len)."""
    bins: list[list[int]] = [[] for _ in range(NCORES)]
    loads = np.zeros(NCORES, np.int64)
    for i in np.argsort(-lens, kind="stable"):
        open_ = [c for c in range(NCORES) if len(bins[c]) < BL]
        c = min(open_, key=lambda c: loads[c])
        bins[c].append(int(i))
        loads[c] += lens[i]
    # pairwise-swap local search to shave the max bin
    for _ in range(64):
        hi = int(np.argmax(loads))
        best = None
        for lo in range(NCORES):
            if lo == hi:
                continue
            for a in bins[hi]:
                for bb in bins[lo]:
                    d = lens[a] - lens[bb]
                    if d <= 0:
                        continue
                    new_hi, new_lo = loads[hi] - d, loads[lo] + d
                    peak = max(new_hi, new_lo)
                    if peak < loads[hi] and (best is None or peak < best[0]):
                        best = (peak, lo, a, bb)
        if best is None:
            break
        _, lo, a, bb = best
        bins[hi].remove(a)
        bins[lo].remove(bb)
        bins[hi].append(bb)
        bins[lo].append(a)
        d = lens[a] - lens[bb]
        loads[hi] -= d
        loads[lo] += d
    return bins


def kernel(last_hidden_state, pooled_output, position_indices, W, b):
    lhs = np.ascontiguousarray(last_hidden_state, dtype=np.float32)
    pooled = np.ascontiguousarray(pooled_output, dtype=np.float32)
    pos = np.ascontiguousarray(position_indices, dtype=np.int32)
    W = np.ascontiguousarray(W, dtype=np.float32)
    b = np.ascontiguousarray(b, dtype=np.float32)

    starts = pos[:, 0].astype(np.int64)
    lens = (pos[:, 1] - pos[:, 0]).astype(np.int64)
    bins = _assign(lens)
    maxload = max(int(lens[ids].sum()) for ids in bins)
    G = 1
    while G * P < maxload:
        G *= 2
    with_bias = bool(np.any(b != 0))

    WM0, ON0, B0, BW = _cols(G)
    wT = W.reshape(C, KC, P).transpose(2, 1, 0).reshape(P, KC * C)
    in_maps = []
    for ids in bins:
        idx = np.zeros(G * P, np.int32)
        wm = np.zeros((P, BL * G), np.float32)
        r = 0
        for sloc, sid in enumerate(ids):
            L = int(lens[sid])
            rr = np.arange(r, r + L)
            gg, pp = rr // P, rr % P
            idx[gg * P + pp] = sloc * S + int(starts[sid]) + np.arange(L)
            wm[pp, gg * BL + sloc] = 1.0 / L
            r += L
        blob = np.zeros((P, BW), np.float32)
        blob[:, PT0:PT0 + HC * BL] = (
            pooled[ids].reshape(BL, HC, P).transpose(2, 1, 0).reshape(P, -1))
        blob[:, WT0:WT0 + KC * C] = wT
        blob[:, WM0:WM0 + BL * G] = wm
        blob[0, ON0:ON0 + BL] = 1.0
        blob[0, B0:B0 + C] = b
        in_maps.append({
            "lhs": lhs[ids].reshape(BL * S, H),
            "idx": idx.reshape(G, P).T.copy(),
            "blob": blob.astype(ml_dtypes.bfloat16),
        })

    if RUN_KWARGS:
        # profiling path (test.py sets trace=True)
        res = run_bass_kernel_spmd(_get(G, with_bias), in_maps,
                                   core_ids=list(range(NCORES)),
                                   **RUN_KWARGS)
        global LAST_RESULT
        LAST_RESULT = res
        results = res.results
    else:
        results = _run_fast(G, with_bias, in_maps)

    out = np.zeros((B, C), np.float32)
    for cid, ids in enumerate(bins):
        out[ids] = results[cid]["out"]
    return out


# Cached-jit fast path: run_bass_kernel_spmd re-jits its PJRT wrapper on
# every call (~17s), so repeated kernel() calls would pay the full XLA +
# neuronx-cc pipeline each time.  This replicates bass2jax.run_bass_via_pjrt
# (multi-core branch) once per G and reuses the compiled executable.
_RUNNER_CACHE: dict = {}


def _get_runner(G, with_bias):
    key = (G, PID, with_bias, WARMUP)
    if key in _RUNNER_CACHE:
        return _RUNNER_CACHE[key]
    import jax
    from jax.experimental.shard_map import shard_map
    from jax.sharding import Mesh, PartitionSpec
    from concourse import bass2jax

    nc = _get(G, with_bias)
    bass2jax.install_neuronx_cc_hook()
    assert nc.dbg_addr is None, "fast path assumes debug-free program"
    partition_name = (nc.partition_id_tensor.name
                      if nc.partition_id_tensor else None)

    in_names, out_names, out_avals = [], [], []
    for alloc in nc.m.functions[0].allocations:
        if not isinstance(alloc, mybir.MemoryLocationSet):
            continue
        name = alloc.memorylocations[0].name
        if alloc.kind == "ExternalInput":
            if name != partition_name:
                in_names.append(name)
        elif alloc.kind == "ExternalOutput":
            shape = tuple(alloc.tensor_shape)
            dtype = mybir.dt.np(alloc.dtype)
            out_names.append(name)
            out_avals.append(jax.core.ShapedArray(shape, dtype))
    n_params = len(in_names)
    n_outs = len(out_avals)
    all_names = in_names + out_names
    if partition_name is not None:
        all_names = all_names + [partition_name]

    def _body(*args):
        operands = list(args)
        if partition_name is not None:
            operands.append(bass2jax.partition_id_tensor())
        outs = bass2jax._bass_exec_p.bind(
            *operands,
            out_avals=tuple(out_avals),
            in_names=tuple(all_names),
            out_names=tuple(out_names),
            lowering_input_output_aliases=(),
            sim_require_finite=True,
            sim_require_nnan=True,
            nc=nc,
        )
        return tuple(outs)

    devices = jax.devices()[:NCORES]
    mesh = Mesh(np.asarray(devices), ("core",))
    specs = (PartitionSpec("core"),) * (n_params + n_outs)
    out_specs = (PartitionSpec("core"),) * n_outs
    donate = (tuple(range(n_params, n_params + n_outs))
              if devices[0].platform != "cpu" else ())
    sharded = jax.jit(
        shard_map(_body, mesh=mesh, in_specs=specs, out_specs=out_specs,
                  check_rep=False),
        donate_argnums=donate,
        keep_unused=True,
    )
    runner = (sharded, in_names, out_names, out_avals, n_params)
    _RUNNER_CACHE[key] = runner
    return runner


def _run_fast(G, with_bias, in_maps):
    sharded, in_names, out_names, out_avals, n_params = _get_runner(
        G, with_bias)
    concat_in = [
        np.concatenate([np.asarray(in_maps[c][k]) for c in range(NCORES)],
                       axis=0)
        for k in in_names
    ]
    concat_zeros = [
        np.zeros((NCORES * a.shape[0], *a.shape[1:]), a.dtype)
        for a in out_avals
    ]
    out_arrs = sharded(*concat_in, *concat_zeros)
    return [
        {name: np.asarray(out_arrs[i]).reshape(NCORES, *out_avals[i].shape)[c]
         for i, name in enumerate(out_names)}
        for c in range(NCORES)
    ]


# test/bench hooks (harness just calls kernel(); these stay default)
RUN_KWARGS: dict = {}
LAST_RESULT = None
